# revision 7
# baseline (speedup 1.0000x reference)
"""Tensor-parallel GQA attention (sigmoid-gated) for Trainium2, 8 NeuronCores.

Problem: B=2, S=2048, D=2048, H=32 q-heads, KV=8 kv-heads, HD=64 (GQA groups=4),
RoPE on q/k, full (non-causal) softmax, sigmoid(gate) output gating, out proj.

Sharding (tensor-parallel over heads): core c owns q-heads 4c..4c+3, kv-head c,
the matching 256 q-cols + 256 gate-cols of Wq, 64-col slices of Wk/Wv, and rows
256c:256c+256 of Wo. Each core computes a full [B*S, D] partial of the output
projection; the host sums the 8 partials.

Per-core pipeline (all matmuls in float32r = full-rate reduced-precision fp32):
  A) projections psum[m,t] += W[d,m].T @ hsT[d,t]  (hsT host-pre-transposed so
     the contraction dim d sits on partitions).  Gate columns get sigmoid
     applied at eviction and round-trip through DRAM (SBUF pressure).
  B) attention in scoresT orientation: scoresT[j,i] = kT[hd,j].T @ qT[hd,i]
     (both [hd,t] slices fall out of stage A untransposed), exp on ACT with no
     max-subtraction (|scores| <= ~6 measured), then
     attnT[hd,i] += v1[j, hd|1].T @ expT[j,i] -- v1's appended ones-column
     accumulates the softmax denominators in psum row 64 for free.
     1/denom is broadcast across partitions with a K=1 ones-matmul.
  C) out[t,dout] += attnGT[m,t].T @ Wo[m,dout] partial, DMA'd out.
"""

import sys

sys.path.insert(0, "/opt/trn_rl_repo")

import numpy as np

import concourse.bass as bass  # noqa: F401
import concourse.mybir as mybir
import concourse.tile as tile
from concourse import bacc
from concourse.bass_utils import run_bass_kernel_spmd

F32 = mybir.dt.float32
F32R = mybir.dt.float32r
AF = mybir.ActivationFunctionType

P = 128
B, S, D = 2, 2048, 2048
T = B * S                  # 4096 token rows (batch folded)
H, KV, HD = 32, 8, 64
HH = HD // 2
NCORES = 8
NH = H // NCORES           # 4 q-heads per core
MQ = NH * HD               # 256 q-cols per core
DC = D // P                # 16 contraction chunks
TCH = 512                  # moving-dim chunk
NTCH = T // TCH            # 8
SJ = S // P                # 16 key chunks per batch
NT = T // P                # 32 t-tiles


def build_nc():
    nc = bacc.Bacc("TRN2", target_bir_lowering=False, debug=False)

    hsT = nc.dram_tensor("hsT", [D, T], F32R, kind="ExternalInput")
    wqg = nc.dram_tensor("wqg", [D, 2 * MQ], F32R, kind="ExternalInput")
    wkv = nc.dram_tensor("wkv", [D, 2 * HD], F32R, kind="ExternalInput")
    wo = nc.dram_tensor("wo", [MQ, D], F32R, kind="ExternalInput")
    # rope tables, [128, S] with rows duplicated (row p holds entry p % 64)
    cq = nc.dram_tensor("cq", [P, S], F32, kind="ExternalInput")   # cos/8
    sq = nc.dram_tensor("sq", [P, S], F32, kind="ExternalInput")   # signed sin/8
    ck = nc.dram_tensor("ck", [P, S], F32, kind="ExternalInput")
    sk = nc.dram_tensor("sk", [P, S], F32, kind="ExternalInput")
    identd = nc.dram_tensor("ident", [HD, HD], F32R, kind="ExternalInput")
    onesd = nc.dram_tensor("ones", [P, B * SJ], F32R, kind="ExternalInput")
    out = nc.dram_tensor("out", [T, D], F32, kind="ExternalOutput")

    gbuf = nc.dram_tensor("gbuf", [P, 2, T], F32)  # sigmoid(gate), internal

    hsT3 = hsT.ap().rearrange("(o p) t -> p o t", p=P)   # [128, 16, 4096]
    wqg3 = wqg.ap().rearrange("(o p) m -> p o m", p=P)   # [128, 16, 512]
    wkv3 = wkv.ap().rearrange("(o p) m -> p o m", p=P)   # [128, 16, 128]
    wo3 = wo.ap().rearrange("(o p) n -> p o n", p=P)     # [128, 2, 2048]

    with tile.TileContext(nc) as tc:
        with (
            tc.tile_pool(name="const", bufs=1) as const,
            tc.tile_pool(name="big", bufs=1) as big,
            tc.tile_pool(name="evc", bufs=4) as evc,
        ):
            # ---- small constants ----
            ident_sb = const.tile([HD, HD], F32R)
            ones_sb = const.tile([1, HD], F32R)
            nc.sync.dma_start(ident_sb[:], identd.ap())
            nc.sync.dma_start(
                ones_sb[:], onesd.ap()[0:HD, 0:1].rearrange("x y -> y x")
            )

            # ---- persistent activations ----
            # per-head q: head h canonical at partition half h%2, duplicated to
            # the other half after rope (for row-tiled concurrent scores)
            qT2_sb = big.tile([P, NH, T], F32R)
            kv2_sb = big.tile([P, T], F32R)      # rows 0:64 roped kT; 64:128 dup
            v1_sb = big.tile([P, B * SJ, HD + 1], F32R)  # v rows | ones col
            attnG_sb = big.tile([P, 2, T], F32R)

            # ---- stage A: projections ----
            with (
                nc.named_scope("stageA"),
                tc.tile_pool(name="wpool", bufs=1) as wpool,
                tc.tile_pool(name="hst", bufs=10) as hst_pool,
                tc.tile_pool(name="vst", bufs=2) as vst,
                tc.tile_pool(name="ps512", bufs=6, space="PSUM") as ps512,
                tc.tile_pool(name="psvt", bufs=2, space="PSUM") as psvt,
            ):
                wqg_sb = wpool.tile([P, DC, 2 * MQ], F32R)
                wkv_sb = wpool.tile([P, DC, 2 * HD], F32R)
                nc.sync.dma_start(wqg_sb[:], wqg3)
                nc.sync.dma_start(wkv_sb[:], wkv3)
                for tci in range(NTCH):
                    ts = slice(tci * TCH, (tci + 1) * TCH)
                    hts = []
                    for dc in range(DC):
                        ht = hst_pool.tile([P, TCH], F32R, tag="hst")
                        nc.sync.dma_start(ht[:], hsT3[:, dc, ts])
                        hts.append(ht)
                    pss = [ps512.tile([P, TCH], F32, tag="ps512", name=f"psA{_m}") for _m in range(5)]
                    for dc in range(DC):
                        for mt in range(5):  # 0: kv, 1-2: q, 3-4: gate
                            if mt == 0:
                                w = wkv_sb[:, dc, :]
                            else:
                                w = wqg_sb[:, dc, (mt - 1) * P:mt * P]
                            nc.tensor.matmul(
                                pss[mt][:],
                                lhsT=w,
                                rhs=hts[dc][:],
                                start=(dc == 0),
                                stop=(dc == DC - 1),
                            )
                    # evictions
                    nc.vector.tensor_copy(kv2_sb[0:HD, ts], pss[0][0:HD, :])
                    vstg = vst.tile([HD, TCH], F32R, tag="vst")
                    nc.vector.tensor_copy(vstg[:], pss[0][HD:P, :])
                    for mo in range(2):
                        nc.scalar.copy(
                            qT2_sb[0:HD, 2 * mo, ts], pss[1 + mo][0:HD, :]
                        )
                        nc.scalar.copy(
                            qT2_sb[HD:P, 2 * mo + 1, ts], pss[1 + mo][HD:P, :]
                        )
                    for mo in range(2):
                        ev = evc.tile([P, TCH], F32, tag="ev")
                        nc.scalar.activation(ev[:], pss[3 + mo][:], AF.Sigmoid)
                        nc.sync.dma_start(gbuf.ap()[:, mo, ts], ev[:])
                    # v1: transpose the 4 key-chunks of this t-chunk
                    for j4 in range(TCH // P):
                        jc = tci * (TCH // P) + j4
                        vt_ps = psvt.tile([P, HD], F32R, tag="psvt")
                        nc.tensor.transpose(
                            vt_ps[:],
                            vstg[:, j4 * P:(j4 + 1) * P],
                            ident_sb[:],
                        )
                        nc.vector.tensor_copy(v1_sb[:, jc, 0:HD], vt_ps[:])
                nc.sync.dma_start(v1_sb[:, :, HD:HD + 1], onesd.ap()[:, :, None])

            # ---- rope ----
            with (
                nc.named_scope("rope"),
                tc.tile_pool(name="tab", bufs=1) as tab,
                tc.tile_pool(name="ropep", bufs=1) as rope_pool,
            ):
                cq_sb = tab.tile([P, S], F32)
                sq_sb = tab.tile([P, S], F32)
                ck_sb = tab.tile([P, S], F32)
                sk_sb = tab.tile([P, S], F32)
                nc.sync.dma_start(cq_sb[:], cq.ap())
                nc.sync.dma_start(sq_sb[:], sq.ap())
                nc.sync.dma_start(ck_sb[:], ck.ap())
                nc.sync.dma_start(sk_sb[:], sk.ap())

                def rope(x, hp, ctab, stab):
                    # x: [64, S] f32r slice at partition base hp; in-place
                    # x = x*cos + rot_half(x)*sin_signed
                    rot = rope_pool.tile([P, S], F32, tag="rot")
                    r = rot[hp:hp + HD, :]
                    nc.vector.tensor_copy(rot[hp:hp + HH, :], x[HH:HD, :])
                    nc.vector.tensor_copy(rot[hp + HH:hp + HD, :], x[0:HH, :])
                    nc.vector.tensor_mul(out=r, in0=r, in1=stab[hp:hp + HD, :])
                    nc.vector.tensor_mul(out=x, in0=x, in1=ctab[hp:hp + HD, :])
                    nc.vector.tensor_add(out=x, in0=x, in1=r)
                for b in range(B):
                    bs = slice(b * S, (b + 1) * S)
                    rope(kv2_sb[0:HD, bs], 0, ck_sb, sk_sb)
                for h in range(NH):
                    hp = (h % 2) * HD
                    for b in range(B):
                        bs = slice(b * S, (b + 1) * S)
                        rope(qT2_sb[hp:hp + HD, h, bs], hp, cq_sb, sq_sb)
                # duplicate roped kT into rows 64:128 (for row-tiled scores)
                nc.vector.tensor_copy(kv2_sb[HD:P, :], kv2_sb[0:HD, :])
                # duplicate each head's roped q into the other partition half
                for h in range(NH):
                    hp = (h % 2) * HD
                    op = HD - hp
                    nc.sync.dma_start(
                        qT2_sb[op:op + HD, h, :].bitcast(F32),
                        qT2_sb[hp:hp + HD, h, :].bitcast(F32),
                    )

            # ---- stage B: attention ----
            with (
                nc.named_scope("stageB"),
                tc.tile_pool(name="exp", bufs=3) as exp_pool,
                tc.tile_pool(name="small", bufs=2) as small,
                tc.tile_pool(name="pssc", bufs=2, space="PSUM") as pssc,
                tc.tile_pool(name="psat", bufs=4, space="PSUM") as psat,
            ):
                for b in range(B):
                    for h in range(NH):
                        hp = (h % 2) * HD
                        ho = h // 2
                        # sigmoid(gate) slice for this (b, h), DMA'd to base hp
                        sgt = small.tile([P, S], F32, tag="sgt")
                        nc.sync.dma_start(
                            sgt[hp:hp + HD, :],
                            gbuf.ap()[hp:hp + HD, ho, b * S:(b + 1) * S],
                        )
                        a_ps = [
                            psat.tile([P, TCH], F32, tag="psat", name=f"psat{_g}")
                            for _g in range(4)
                        ]

                        def scores_exp(jc, h=h, b=b):
                            jsl = slice(b * S + jc * P, b * S + (jc + 1) * P)
                            exs = []
                            for ih in range(2):
                                s_ps = pssc.tile([P, 2 * TCH], F32, tag="pssc")
                                for ii in range(2):
                                    # ii=0 in PE row-group 0-63, ii=1 in 64-127;
                                    # different PSUM banks -> run concurrently
                                    rp = ii * HD
                                    i0 = b * S + (ih * 2 + ii) * TCH
                                    nc.tensor.matmul(
                                        s_ps[:, ii * TCH:(ii + 1) * TCH],
                                        lhsT=kv2_sb[rp:rp + HD, jsl],
                                        rhs=qT2_sb[rp:rp + HD, h, i0:i0 + TCH],
                                        start=True,
                                        stop=True,
                                    )
                                ex = exp_pool.tile([P, 2 * TCH], F32R, tag="exp")
                                nc.scalar.activation(ex[:], s_ps[:], AF.Exp)
                                exs.append(ex)
                            return exs

                        def attn_acc(jc, exs, b=b):
                            for ih in range(2):
                                for ii in range(2):
                                    nc.tensor.matmul(
                                        a_ps[ih * 2 + ii][0:HD + 1, :],
                                        lhsT=v1_sb[:, b * SJ + jc, :],
                                        rhs=exs[ih][:, ii * TCH:(ii + 1) * TCH],
                                        start=(jc == 0),
                                        stop=(jc == SJ - 1),
                                    )

                        prev = scores_exp(0)
                        for jc in range(1, SJ):
                            cur = scores_exp(jc)
                            attn_acc(jc - 1, prev)
                            prev = cur
                        attn_acc(SJ - 1, prev)

                        # ---- normalize + gate ----
                        for seg in range(4):
                            osl = slice(b * S + seg * TCH, b * S + (seg + 1) * TCH)
                            den = small.tile([1, TCH], F32R, tag="den")
                            nc.vector.tensor_copy(den[:], a_ps[seg][HD:HD + 1, :])
                            bc_ps = pssc.tile([P, 2 * TCH], F32, tag="pssc")
                            nc.tensor.matmul(
                                bc_ps[0:HD, 0:TCH],
                                lhsT=ones_sb[:],
                                rhs=den[:],
                                start=True,
                                stop=True,
                            )
                            rcp = small.tile([P, TCH], F32, tag="rcp")
                            nc.vector.reciprocal(rcp[0:HD, :], bc_ps[0:HD, 0:TCH])
                            if hp:
                                nc.vector.tensor_copy(
                                    rcp[HD:P, :], rcp[0:HD, :]
                                )
                            ag = attnG_sb[hp:hp + HD, ho, osl]
                            nc.vector.tensor_mul(
                                out=ag,
                                in0=a_ps[seg][0:HD, :],
                                in1=rcp[hp:hp + HD, :],
                            )
                            nc.vector.tensor_mul(
                                out=ag,
                                in0=ag,
                                in1=sgt[hp:hp + HD, seg * TCH:(seg + 1) * TCH],
                            )

            # ---- stage C: output projection (partial) ----
            with (
                nc.named_scope("stageC"),
                tc.tile_pool(name="wop", bufs=1) as wop,
                tc.tile_pool(name="psC", bufs=4, space="PSUM") as psC,
            ):
                wo_sb = wop.tile([P, 2, D], F32R)
                nc.sync.dma_start(wo_sb[:], wo3)
                for tt in range(NT):
                    tsl = slice(tt * P, (tt + 1) * P)
                    for oc in range(D // TCH):
                        ps = psC.tile([P, TCH], F32, tag="psC")
                        for mc in range(2):
                            nc.tensor.matmul(
                                ps[:],
                                lhsT=attnG_sb[:, mc, tsl],
                                rhs=wo_sb[:, mc, oc * TCH:(oc + 1) * TCH],
                                start=(mc == 0),
                                stop=(mc == 1),
                            )
                        ev = evc.tile([P, TCH], F32, tag="ev")
                        nc.vector.tensor_copy(ev[:], ps[:])
                        nc.sync.dma_start(
                            out.ap()[tsl, oc * TCH:(oc + 1) * TCH], ev[:]
                        )

    nc.compile()
    return nc


_NC_CACHE = None


def _get_nc():
    global _NC_CACHE
    if _NC_CACHE is None:
        _NC_CACHE = build_nc()
    return _NC_CACHE


def _dup_rows(tab64):
    """[64, S] -> [128, S] with both partition halves holding the table."""
    return np.ascontiguousarray(np.concatenate([tab64, tab64], axis=0))


def _prep_inputs(hidden_states, cos, sin, Wq, Wk, Wv, Wo):
    hs = np.asarray(hidden_states, dtype=np.float32)
    cos = np.asarray(cos, dtype=np.float32)
    sin = np.asarray(sin, dtype=np.float32)
    Wq = np.asarray(Wq, dtype=np.float32)
    Wk = np.asarray(Wk, dtype=np.float32)
    Wv = np.asarray(Wv, dtype=np.float32)
    Wo = np.asarray(Wo, dtype=np.float32)

    hsT = np.ascontiguousarray(hs.reshape(T, D).T)

    cosT = cos.T                                     # [64, S]
    sinT = sin.T
    sin_signed = np.concatenate([-sinT[:HH], sinT[HH:]], axis=0)
    scale = np.float32(1.0 / np.sqrt(HD))
    common = {
        "hsT": hsT,
        "cq": _dup_rows(cosT * scale),
        "sq": _dup_rows(sin_signed * scale),
        "ck": _dup_rows(cosT),
        "sk": _dup_rows(sin_signed),
        "ident": np.eye(HD, dtype=np.float32),
        "ones": np.ones((P, B * SJ), np.float32),
    }
    in_maps = []
    for c in range(NCORES):
        qcols = Wq[:, c * MQ:(c + 1) * MQ]
        gcols = Wq[:, H * HD + c * MQ: H * HD + (c + 1) * MQ]
        in_maps.append(
            {
                **common,
                "wqg": np.ascontiguousarray(
                    np.concatenate([qcols, gcols], axis=1)
                ),
                "wkv": np.ascontiguousarray(
                    np.concatenate(
                        [Wk[:, c * HD:(c + 1) * HD], Wv[:, c * HD:(c + 1) * HD]],
                        axis=1,
                    )
                ),
                "wo": np.ascontiguousarray(Wo[c * MQ:(c + 1) * MQ, :]),
            }
        )
    return in_maps


def kernel(hidden_states, cos, sin, Wq, Wk, Wv, Wo, _trace=False, _trace_kwargs=None):
    nc = _get_nc()
    in_maps = _prep_inputs(hidden_states, cos, sin, Wq, Wk, Wv, Wo)
    res = run_bass_kernel_spmd(
        nc, in_maps, list(range(NCORES)), trace=_trace, **(_trace_kwargs or {})
    )
    total = res.results[0]["out"].astype(np.float32).copy()
    for c in range(1, NCORES):
        total += res.results[c]["out"]
    out = total.reshape(B, S, D)
    if _trace:
        kernel._last_results = res
    return out


# revision 8
# speedup vs baseline: 1.6169x; 1.6169x over previous
"""Tensor-parallel GQA attention (sigmoid-gated) for Trainium2, 8 NeuronCores.

Problem: B=2, S=2048, D=2048, H=32 q-heads, KV=8 kv-heads, HD=64 (GQA groups=4),
RoPE on q/k, full (non-causal) softmax, sigmoid(gate) output gating, out proj.

Sharding (tensor-parallel over heads): core c owns q-heads 4c..4c+3, kv-head c,
the matching 256 q-cols + 256 gate-cols of Wq, 64-col slices of Wk/Wv, and rows
256c:256c+256 of Wo. Each core computes a full [B*S, D] partial of the output
projection; the host sums the 8 partials.

Per-core pipeline (all matmuls in float32r = full-rate reduced-precision fp32):
  A) projections psum[m,t] += W[d,m].T @ hsT[d,t]  (hsT host-pre-transposed so
     the contraction dim d sits on partitions).  Gate columns get sigmoid
     applied at eviction and round-trip through DRAM (SBUF pressure).
  B) attention in scoresT orientation: scoresT[j,i] = kT[hd,j].T @ qT[hd,i]
     (both [hd,t] slices fall out of stage A untransposed), exp on ACT with no
     max-subtraction (|scores| <= ~6 measured), then
     attnT[hd,i] += v1[j, hd|1].T @ expT[j,i] -- v1's appended ones-column
     accumulates the softmax denominators in psum row 64 for free.
     1/denom is broadcast across partitions with a K=1 ones-matmul.
  C) out[t,dout] += attnGT[m,t].T @ Wo[m,dout] partial, DMA'd out.
"""

import sys

sys.path.insert(0, "/opt/trn_rl_repo")

import numpy as np

import concourse.bass as bass  # noqa: F401
import concourse.mybir as mybir
import concourse.tile as tile
from concourse import bacc
from concourse.bass_utils import run_bass_kernel_spmd

F32 = mybir.dt.float32
F32R = mybir.dt.float32r
AF = mybir.ActivationFunctionType

P = 128
B, S, D = 2, 2048, 2048
T = B * S                  # 4096 token rows (batch folded)
H, KV, HD = 32, 8, 64
HH = HD // 2
NCORES = 8
NH = H // NCORES           # 4 q-heads per core
MQ = NH * HD               # 256 q-cols per core
DC = D // P                # 16 contraction chunks
TCH = 512                  # moving-dim chunk
NTCH = T // TCH            # 8
SJ = S // P                # 16 key chunks per batch
NT = T // P                # 32 t-tiles


def build_nc():
    nc = bacc.Bacc("TRN2", target_bir_lowering=False, debug=False)

    hsT = nc.dram_tensor("hsT", [D, T], F32R, kind="ExternalInput")
    wqg = nc.dram_tensor("wqg", [D, 2 * MQ], F32R, kind="ExternalInput")
    wkv = nc.dram_tensor("wkv", [D, 2 * HD], F32R, kind="ExternalInput")
    wo = nc.dram_tensor("wo", [MQ, D], F32R, kind="ExternalInput")
    # rope tables, [128, S] with rows duplicated (row p holds entry p % 64)
    cq = nc.dram_tensor("cq", [P, S], F32, kind="ExternalInput")   # cos/8
    sq = nc.dram_tensor("sq", [P, S], F32, kind="ExternalInput")   # signed sin/8
    ck = nc.dram_tensor("ck", [P, S], F32, kind="ExternalInput")
    sk = nc.dram_tensor("sk", [P, S], F32, kind="ExternalInput")
    identd = nc.dram_tensor("ident", [HD, HD], F32R, kind="ExternalInput")
    onesd = nc.dram_tensor("ones", [P, B * SJ], F32R, kind="ExternalInput")
    out = nc.dram_tensor("out", [T, D], F32, kind="ExternalOutput")

    gbuf = nc.dram_tensor("gbuf", [P, 2, T], F32)  # sigmoid(gate), internal

    hsT3 = hsT.ap().rearrange("(o p) t -> p o t", p=P)   # [128, 16, 4096]
    wqg3 = wqg.ap().rearrange("(o p) m -> p o m", p=P)   # [128, 16, 512]
    wkv3 = wkv.ap().rearrange("(o p) m -> p o m", p=P)   # [128, 16, 128]
    wo3 = wo.ap().rearrange("(o p) n -> p o n", p=P)     # [128, 2, 2048]

    with tile.TileContext(nc) as tc:
        with (
            tc.tile_pool(name="const", bufs=1) as const,
            tc.tile_pool(name="big", bufs=1) as big,
            tc.tile_pool(name="evc", bufs=4) as evc,
        ):
            # ---- small constants ----
            ident_sb = const.tile([HD, HD], F32R)
            ones_sb = const.tile([1, HD], F32R)
            nc.sync.dma_start(ident_sb[:], identd.ap())
            nc.sync.dma_start(
                ones_sb[:], onesd.ap()[0:HD, 0:1].rearrange("x y -> y x")
            )

            # ---- persistent activations ----
            # per-head q: head h canonical at partition half h%2, duplicated to
            # the other half after rope (for row-tiled concurrent scores)
            qT2_sb = big.tile([P, NH, T], F32R)
            kv2_sb = big.tile([P, T], F32R)      # rows 0:64 roped kT; 64:128 dup
            v1_sb = big.tile([P, B * SJ, HD + 1], F32R)  # v rows | ones col
            attnG_sb = big.tile([P, 2, T], F32R)

            # ---- stage A: projections ----
            with (
                nc.named_scope("stageA"),
                tc.tile_pool(name="wpool", bufs=1) as wpool,
                tc.tile_pool(name="hst", bufs=10) as hst_pool,
                tc.tile_pool(name="vst", bufs=2) as vst,
                tc.tile_pool(name="ps512", bufs=6, space="PSUM") as ps512,
                tc.tile_pool(name="psvt", bufs=2, space="PSUM") as psvt,
            ):
                wqg_sb = wpool.tile([P, DC, 2 * MQ], F32R)
                wkv_sb = wpool.tile([P, DC, 2 * HD], F32R)
                nc.sync.dma_start(wqg_sb[:], wqg3)
                nc.sync.dma_start(wkv_sb[:], wkv3)
                for tci in range(NTCH):
                    ts = slice(tci * TCH, (tci + 1) * TCH)
                    hts = []
                    for dc in range(DC):
                        ht = hst_pool.tile([P, TCH], F32R, tag="hst")
                        nc.sync.dma_start(ht[:], hsT3[:, dc, ts])
                        hts.append(ht)
                    pss = [ps512.tile([P, TCH], F32, tag="ps512", name=f"psA{_m}") for _m in range(5)]
                    for dc in range(DC):
                        for mt in range(5):  # 0: kv, 1-2: q, 3-4: gate
                            if mt == 0:
                                w = wkv_sb[:, dc, :]
                            else:
                                w = wqg_sb[:, dc, (mt - 1) * P:mt * P]
                            nc.tensor.matmul(
                                pss[mt][:],
                                lhsT=w,
                                rhs=hts[dc][:],
                                start=(dc == 0),
                                stop=(dc == DC - 1),
                            )
                    # evictions
                    nc.vector.tensor_copy(kv2_sb[0:HD, ts], pss[0][0:HD, :])
                    vstg = vst.tile([HD, TCH], F32R, tag="vst")
                    nc.vector.tensor_copy(vstg[:], pss[0][HD:P, :])
                    for mo in range(2):
                        nc.scalar.copy(
                            qT2_sb[0:HD, 2 * mo, ts], pss[1 + mo][0:HD, :]
                        )
                        nc.scalar.copy(
                            qT2_sb[HD:P, 2 * mo + 1, ts], pss[1 + mo][HD:P, :]
                        )
                    for mo in range(2):
                        ev = evc.tile([P, TCH], F32, tag="ev")
                        nc.scalar.activation(ev[:], pss[3 + mo][:], AF.Sigmoid)
                        nc.sync.dma_start(gbuf.ap()[:, mo, ts], ev[:])
                    # v1: transpose the 4 key-chunks of this t-chunk
                    for j4 in range(TCH // P):
                        jc = tci * (TCH // P) + j4
                        vt_ps = psvt.tile([P, HD], F32R, tag="psvt")
                        nc.tensor.transpose(
                            vt_ps[:],
                            vstg[:, j4 * P:(j4 + 1) * P],
                            ident_sb[:],
                        )
                        nc.vector.tensor_copy(v1_sb[:, jc, 0:HD], vt_ps[:])
                nc.sync.dma_start(v1_sb[:, :, HD:HD + 1], onesd.ap()[:, :, None])

            # ---- rope ----
            with (
                nc.named_scope("rope"),
                tc.tile_pool(name="tab", bufs=1) as tab,
                tc.tile_pool(name="ropep", bufs=1) as rope_pool,
            ):
                cq_sb = tab.tile([P, S], F32)
                sq_sb = tab.tile([P, S], F32)
                ck_sb = tab.tile([P, S], F32)
                sk_sb = tab.tile([P, S], F32)
                nc.sync.dma_start(cq_sb[:], cq.ap())
                nc.sync.dma_start(sq_sb[:], sq.ap())
                nc.sync.dma_start(ck_sb[:], ck.ap())
                nc.sync.dma_start(sk_sb[:], sk.ap())

                def rope(x, hp, ctab, stab):
                    # x: [64, S] f32r slice at partition base hp; in-place
                    # x = x*cos + rot_half(x)*sin_signed
                    rot = rope_pool.tile([P, S], F32, tag="rot")
                    r = rot[hp:hp + HD, :]
                    nc.vector.tensor_copy(rot[hp:hp + HH, :], x[HH:HD, :])
                    nc.vector.tensor_copy(rot[hp + HH:hp + HD, :], x[0:HH, :])
                    nc.vector.tensor_mul(out=r, in0=r, in1=stab[hp:hp + HD, :])
                    nc.vector.tensor_mul(out=x, in0=x, in1=ctab[hp:hp + HD, :])
                    nc.vector.tensor_add(out=x, in0=x, in1=r)
                for b in range(B):
                    bs = slice(b * S, (b + 1) * S)
                    rope(kv2_sb[0:HD, bs], 0, ck_sb, sk_sb)
                for h in range(NH):
                    hp = (h % 2) * HD
                    for b in range(B):
                        bs = slice(b * S, (b + 1) * S)
                        rope(qT2_sb[hp:hp + HD, h, bs], hp, cq_sb, sq_sb)
                # duplicate roped kT into rows 64:128 (for row-tiled scores)
                nc.vector.tensor_copy(kv2_sb[HD:P, :], kv2_sb[0:HD, :])
                # duplicate each head's roped q into the other partition half
                for h in range(NH):
                    hp = (h % 2) * HD
                    op = HD - hp
                    nc.sync.dma_start(
                        qT2_sb[op:op + HD, h, :].bitcast(F32),
                        qT2_sb[hp:hp + HD, h, :].bitcast(F32),
                    )

            # ---- stage B: attention ----
            with (
                nc.named_scope("stageB"),
                tc.tile_pool(name="exp", bufs=3) as exp_pool,
                tc.tile_pool(name="small", bufs=2) as small,
                tc.tile_pool(name="pssc", bufs=2, space="PSUM") as pssc,
                tc.tile_pool(name="psat", bufs=4, space="PSUM") as psat,
            ):
                for b in range(B):
                    for h in range(NH):
                        hp = (h % 2) * HD
                        ho = h // 2
                        # sigmoid(gate) slice for this (b, h), DMA'd to base hp
                        sgt = small.tile([P, S], F32, tag="sgt")
                        nc.sync.dma_start(
                            sgt[hp:hp + HD, :],
                            gbuf.ap()[hp:hp + HD, ho, b * S:(b + 1) * S],
                        )
                        a_ps = [
                            psat.tile([P, TCH], F32, tag="psat", name=f"psat{_g}")
                            for _g in range(4)
                        ]

                        def scores_exp(jc, h=h, b=b):
                            jsl = slice(b * S + jc * P, b * S + (jc + 1) * P)
                            exs = []
                            for ih in range(2):
                                s_ps = pssc.tile([P, 2 * TCH], F32, tag="pssc")
                                for ii in range(2):
                                    rp = (h % 2) * HD
                                    i0 = b * S + (ih * 2 + ii) * TCH
                                    nc.tensor.matmul(
                                        s_ps[:, ii * TCH:(ii + 1) * TCH],
                                        lhsT=kv2_sb[rp:rp + HD, jsl],
                                        rhs=qT2_sb[rp:rp + HD, h, i0:i0 + TCH],
                                        start=True,
                                        stop=True,
                                    )
                                ex = exp_pool.tile([P, 2 * TCH], F32R, tag="exp")
                                nc.scalar.activation(ex[:], s_ps[:], AF.Exp)
                                exs.append(ex)
                            return exs

                        def attn_acc(jc, exs, b=b):
                            for ih in range(2):
                                for ii in range(2):
                                    nc.tensor.matmul(
                                        a_ps[ih * 2 + ii][0:HD + 1, :],
                                        lhsT=v1_sb[:, b * SJ + jc, :],
                                        rhs=exs[ih][:, ii * TCH:(ii + 1) * TCH],
                                        start=(jc == 0),
                                        stop=(jc == SJ - 1),
                                    )

                        prev = scores_exp(0)
                        for jc in range(1, SJ):
                            cur = scores_exp(jc)
                            attn_acc(jc - 1, prev)
                            prev = cur
                        attn_acc(SJ - 1, prev)

                        # ---- normalize + gate ----
                        for seg in range(4):
                            osl = slice(b * S + seg * TCH, b * S + (seg + 1) * TCH)
                            den = small.tile([1, TCH], F32R, tag="den")
                            nc.vector.tensor_copy(den[:], a_ps[seg][HD:HD + 1, :])
                            bc_ps = pssc.tile([P, 2 * TCH], F32, tag="pssc")
                            nc.tensor.matmul(
                                bc_ps[0:HD, 0:TCH],
                                lhsT=ones_sb[:],
                                rhs=den[:],
                                start=True,
                                stop=True,
                            )
                            rcp = small.tile([P, TCH], F32, tag="rcp")
                            nc.vector.reciprocal(rcp[0:HD, :], bc_ps[0:HD, 0:TCH])
                            if hp:
                                nc.vector.tensor_copy(
                                    rcp[HD:P, :], rcp[0:HD, :]
                                )
                            ag = attnG_sb[hp:hp + HD, ho, osl]
                            nc.vector.tensor_mul(
                                out=ag,
                                in0=a_ps[seg][0:HD, :],
                                in1=rcp[hp:hp + HD, :],
                            )
                            nc.vector.tensor_mul(
                                out=ag,
                                in0=ag,
                                in1=sgt[hp:hp + HD, seg * TCH:(seg + 1) * TCH],
                            )

            # ---- stage C: output projection (partial) ----
            with (
                nc.named_scope("stageC"),
                tc.tile_pool(name="wop", bufs=1) as wop,
                tc.tile_pool(name="psC", bufs=4, space="PSUM") as psC,
            ):
                wo_sb = wop.tile([P, 2, D], F32R)
                nc.sync.dma_start(wo_sb[:], wo3)
                for tt in range(NT):
                    tsl = slice(tt * P, (tt + 1) * P)
                    for oc in range(D // TCH):
                        ps = psC.tile([P, TCH], F32, tag="psC")
                        for mc in range(2):
                            nc.tensor.matmul(
                                ps[:],
                                lhsT=attnG_sb[:, mc, tsl],
                                rhs=wo_sb[:, mc, oc * TCH:(oc + 1) * TCH],
                                start=(mc == 0),
                                stop=(mc == 1),
                            )
                        ev = evc.tile([P, TCH], F32, tag="ev")
                        nc.vector.tensor_copy(ev[:], ps[:])
                        nc.sync.dma_start(
                            out.ap()[tsl, oc * TCH:(oc + 1) * TCH], ev[:]
                        )

    nc.compile()
    return nc


_NC_CACHE = None


def _get_nc():
    global _NC_CACHE
    if _NC_CACHE is None:
        _NC_CACHE = build_nc()
    return _NC_CACHE


def _dup_rows(tab64):
    """[64, S] -> [128, S] with both partition halves holding the table."""
    return np.ascontiguousarray(np.concatenate([tab64, tab64], axis=0))


def _prep_inputs(hidden_states, cos, sin, Wq, Wk, Wv, Wo):
    hs = np.asarray(hidden_states, dtype=np.float32)
    cos = np.asarray(cos, dtype=np.float32)
    sin = np.asarray(sin, dtype=np.float32)
    Wq = np.asarray(Wq, dtype=np.float32)
    Wk = np.asarray(Wk, dtype=np.float32)
    Wv = np.asarray(Wv, dtype=np.float32)
    Wo = np.asarray(Wo, dtype=np.float32)

    hsT = np.ascontiguousarray(hs.reshape(T, D).T)

    cosT = cos.T                                     # [64, S]
    sinT = sin.T
    sin_signed = np.concatenate([-sinT[:HH], sinT[HH:]], axis=0)
    scale = np.float32(1.0 / np.sqrt(HD))
    common = {
        "hsT": hsT,
        "cq": _dup_rows(cosT * scale),
        "sq": _dup_rows(sin_signed * scale),
        "ck": _dup_rows(cosT),
        "sk": _dup_rows(sin_signed),
        "ident": np.eye(HD, dtype=np.float32),
        "ones": np.ones((P, B * SJ), np.float32),
    }
    in_maps = []
    for c in range(NCORES):
        qcols = Wq[:, c * MQ:(c + 1) * MQ]
        gcols = Wq[:, H * HD + c * MQ: H * HD + (c + 1) * MQ]
        in_maps.append(
            {
                **common,
                "wqg": np.ascontiguousarray(
                    np.concatenate([qcols, gcols], axis=1)
                ),
                "wkv": np.ascontiguousarray(
                    np.concatenate(
                        [Wk[:, c * HD:(c + 1) * HD], Wv[:, c * HD:(c + 1) * HD]],
                        axis=1,
                    )
                ),
                "wo": np.ascontiguousarray(Wo[c * MQ:(c + 1) * MQ, :]),
            }
        )
    return in_maps


def kernel(hidden_states, cos, sin, Wq, Wk, Wv, Wo, _trace=False, _trace_kwargs=None):
    nc = _get_nc()
    in_maps = _prep_inputs(hidden_states, cos, sin, Wq, Wk, Wv, Wo)
    res = run_bass_kernel_spmd(
        nc, in_maps, list(range(NCORES)), trace=_trace, **(_trace_kwargs or {})
    )
    total = res.results[0]["out"].astype(np.float32).copy()
    for c in range(1, NCORES):
        total += res.results[c]["out"]
    out = total.reshape(B, S, D)
    if _trace:
        kernel._last_results = res
    return out
